# revision 30
# baseline (speedup 1.0000x reference)
"""Trainium2 Bass kernel for nn_CorrelationLayer (441-displacement cost volume).

result[k, i, j] = sum_c f1[c, i, j] * pad(f2)[c, i + dy_k, j + dx_k]
with (dy, dx) in {0, 2, ..., 40}^2, H, W = 48, 64, C = 128, pad D = 20.

Strategy (column-parity split + valid-window trimming)
------------------------------------------------------
Displacements are stride-2 in both axes, so f1 column j only ever
correlates with f2 columns of the SAME parity: the per-row-pair
all-pairs block is 32x32 per parity instead of 64x64 (2x less PE /
copy / DMA than the unsplit scheme).

Sharding: 8 cores = (row parity rp) x (col parity cp) x (half h).
Each core holds 12 f2 rows of parity rp at cp columns, as 3 stationary
tiles ("slots") of 4 rows x 32 cols.  The y-displacement window |r2-i|
<= 20 means slot quads near the volume edge need only 14/18/22 of the
24 same-parity f1 rows; with h=1 cores taking mirrored quads (and f1
rows stored reversed), the slot windows are uniformly 448/576/704
moving columns across all cores -- a single SPMD program computes only
the needed band (1728 instead of 2304 columns).

Scheduling (all latency-bound; every choice paired-A/B-benched on HW):
  - inputs on the two HWDGE rings: aux (= f2 stationary ++ f1 tail) on
    SP (its SDMA starts ~0.5us before ACT's, which is busy with the
    auto-inserted ACT table load), f1 head on ACT.
  - matmul order: the small tail chunks (B1, C1) are gated by aux and
    run in the ~0.5us before the f1 head lands, also warming the PE;
    then C0, B0, A -- the smallest slot (A) last keeps the critical
    tail (cast + DMA issue + ~1.4us HBM-write receipt) short.
  - one full-tile PSUM->SBUF cast per slot, DVE for C/A, ACT for B
    (chunked casts lose ~0.4us fixed overhead per op; matmuls are
    ISA-capped at 512 output columns).
  - two output DMAs: C on SP, B+A merged on ACT; the output stream is
    HBM-write-bound (~220 GB/s aggregate; more queues don't help).
  - no PE warm-up matmuls: the HAM clock ramp needs ~5us of busy PE,
    which a ~2us kernel can never amortize (benched).

The band/diagonal gather and zero-fill is a pure data rearrangement
done on host during unsharding -- all arithmetic happens on device.
"""

import sys
import types

for _p in ("/opt/trn_rl_repo", "/root/.axon_site"):
    if _p not in sys.path:
        sys.path.insert(0, _p)

import ml_dtypes
import numpy as np

BF16 = ml_dtypes.bfloat16

import concourse.bacc as bacc
import concourse.mybir as mybir
from concourse import tile
from concourse import bass_utils
from concourse.bass_utils import run_bass_kernel_spmd

C = 128
H = 48
W = 64
D = 20
ND = 21            # displacements per axis
NCORES = 8
SLOT_COLS = (448, 576, 704)   # moving cols per stationary slot (A, B, C)
SLOT_OFF = (0, 448, 1024)
TOT = 1728                    # total output columns per core
F1COLS = 704                  # f1 moving columns per core (22 rows x 32)


def _ensure_ntff_hook():
    """Register the axon NTFF profile hook if possible (for trace runs)."""
    try:
        import antenv
        if "antenv.axon_hooks" not in sys.modules:
            mod = types.ModuleType("antenv.axon_hooks")
            _h = [None]
            mod.set_axon_ntff_profile_hook = lambda h: _h.__setitem__(0, h)
            mod.get_axon_ntff_profile_hook = lambda: _h[0]
            sys.modules["antenv.axon_hooks"] = mod
            antenv.axon_hooks = mod
        bass_utils.upload_artifacts = lambda tmpdir: "local://" + tmpdir
        from trn_agent_boot.trn_boot import _ntff_profile_via_ctypes
        sys.modules["antenv.axon_hooks"].set_axon_ntff_profile_hook(
            _ntff_profile_via_ctypes("/opt/axon/libaxon_pjrt.so")
        )
    except Exception:
        pass


def build_program():
    nc = bacc.Bacc(None, target_bir_lowering=False)
    aux = nc.declare_dram_parameter("aux", [C, 576], mybir.dt.bfloat16, isOutput=False)
    f1a = nc.declare_dram_parameter("f1a", [C, 512], mybir.dt.bfloat16, isOutput=False)
    mout = nc.declare_dram_parameter("mout", [C, TOT], mybir.dt.bfloat16, isOutput=True)

    with tile.TileContext(nc) as tc:
        with (
            tc.tile_pool(name="in", bufs=1) as in_pool,
            tc.tile_pool(name="msb", bufs=1) as m_pool,
            tc.tile_pool(name="ps", bufs=1, space="PSUM") as ps_pool,
        ):
            aux_sb = in_pool.tile([C, 576], mybir.dt.bfloat16, tag="aux")
            nc.sync.dma_start(out=aux_sb[:], in_=aux[:])
            f1a_sb = in_pool.tile([C, 512], mybir.dt.bfloat16, tag="f1a")
            nc.scalar.dma_start(out=f1a_sb[:], in_=f1a[:])

            def lhsT(x):
                return aux_sb[:, 128 * x : 128 * (x + 1)]

            f1b_sb = aux_sb

            psA = ps_pool.tile([128, 448], mybir.dt.float32, tag="psA")
            psB = ps_pool.tile([128, 576], mybir.dt.float32, tag="psB")
            psC = ps_pool.tile([128, 704], mybir.dt.float32, tag="psC")

            # the small tail matmuls (B1, C1) are gated by aux, which lands
            # ~0.5us before f1a -- run them FIRST to use that gap (and warm
            # the PE), then the f1a-gated big chunks
            nc.tensor.matmul(
                psB[:, 512:576], lhsT(1), f1b_sb[:, 384:448], start=True, stop=True
            )
            nc.tensor.matmul(
                psC[:, 512:704], lhsT(2), f1b_sb[:, 384:576], start=True, stop=True
            )
            nc.tensor.matmul(psC[:, :512], lhsT(2), f1a_sb[:], start=True, stop=True)
            mC = m_pool.tile([128, 704], mybir.dt.bfloat16, tag="mC")
            nc.vector.tensor_copy(mC[:], psC[:])
            nc.sync.dma_start(out=mout[:, 1024:1728], in_=mC[:])

            nc.tensor.matmul(psB[:, :512], lhsT(1), f1a_sb[:], start=True, stop=True)
            mBA = m_pool.tile([128, 1024], mybir.dt.bfloat16, tag="mBA")
            nc.scalar.copy(mBA[:, 448:1024], psB[:])

            nc.tensor.matmul(psA[:, :448], lhsT(0), f1a_sb[:, :448], start=True, stop=True)
            nc.vector.tensor_copy(mBA[:, :448], psA[:])
            nc.scalar.dma_start(out=mout[:, 0:1024], in_=mBA[:])
    nc.compile()
    return nc


_PROGRAM_CACHE = {}


def _get_program():
    if "nc" not in _PROGRAM_CACHE:
        _PROGRAM_CACHE["nc"] = build_program()
    return _PROGRAM_CACHE["nc"]


def _core_def(m):
    """Core m = rp*4 + cp*2 + h -> (rp, cp, h, f2 rows R, f1 rows I, cols J)."""
    rp, cp, h = m // 4, (m // 2) % 2, m % 2
    gs = (0, 1, 2) if h == 0 else (5, 4, 3)
    R = [rp + 8 * g + 2 * rq for g in gs for rq in range(4)]
    S = range(0, 22) if h == 0 else range(23, 1, -1)
    I = [rp + 2 * s for s in S]
    J = [cp + 2 * u for u in range(32)]
    return rp, cp, h, R, I, J


def _shard_inputs(features_1, features_2):
    f1 = np.ascontiguousarray(features_1, dtype=np.float32)
    f2 = np.ascontiguousarray(features_2, dtype=np.float32)
    in_maps = []
    for m in range(NCORES):
        rp, cp, h, R, I, J = _core_def(m)
        f2g = f2[:, R][:, :, J].reshape(C, 384)
        f1g = f1[:, I][:, :, J].reshape(C, F1COLS)
        aux = np.concatenate([f2g, f1g[:, 512:]], axis=1)
        in_maps.append(
            {
                "aux": np.ascontiguousarray(aux).astype(BF16),
                "f1a": np.ascontiguousarray(f1g[:, :512]).astype(BF16),
            }
        )
    return in_maps


def _assemble(results):
    """Gather the stride-2 displacement band out of the per-core blocks
    (pure indexing -- no arithmetic)."""
    M = np.stack(
        [np.asarray(results[m]["mout"]).astype(np.float32) for m in range(NCORES)]
    )  # [8, 128, 1728]

    dy, dxi, i, j = np.ogrid[0:ND, 0:ND, 0:H, 0:W]
    rp = i & 1
    cp = j & 1
    u = j >> 1
    r2 = i + 2 * dy - 20
    v = u + dxi - 10
    valid = (r2 >= 0) & (r2 < H) & (v >= 0) & (v < 32)
    r2c = np.clip(r2, 0, H - 1)
    vc = np.clip(v, 0, 31)
    k = (r2c - rp) >> 1
    g = k // 4
    rq = k % 4
    h = (g >= 3).astype(int)
    x = np.where(h == 0, g, 5 - g)
    s = (i - rp) >> 1
    l = np.where(h == 0, s, 23 - s)
    off = np.array(SLOT_OFF)[x]
    m = rp * 4 + cp * 2 + h
    part = 32 * rq + vc
    col = off + 32 * l + u
    bm, bp, bc = np.broadcast_arrays(m, part, col)
    out = M[bm, bp, bc]
    out[~np.broadcast_to(valid, out.shape)] = 0.0
    return out.reshape(1, ND * ND, H, W)


def kernel(features_1, features_2):
    nc = _get_program()
    in_maps = _shard_inputs(features_1, features_2)
    res = run_bass_kernel_spmd(nc, in_maps, list(range(NCORES)))
    return _assemble(res.results)


def kernel_traced(features_1, features_2, tmpdir=None):
    """Same as kernel() but with NTFF profiling; returns (output, exec_time_ns)."""
    _ensure_ntff_hook()
    nc = _get_program()
    in_maps = _shard_inputs(features_1, features_2)
    res = run_bass_kernel_spmd(
        nc, in_maps, list(range(NCORES)), trace=True, tmpdir=tmpdir
    )
    return _assemble(res.results), res.exec_time_ns


# revision 31
# speedup vs baseline: 1.0947x; 1.0947x over previous
"""Trainium2 Bass kernel for nn_CorrelationLayer (441-displacement cost volume).

result[k, i, j] = sum_c f1[c, i, j] * pad(f2)[c, i + dy_k, j + dx_k]
with (dy, dx) in {0, 2, ..., 40}^2, H, W = 48, 64, C = 128, pad D = 20.

Strategy (column-parity split + valid-window trimming)
------------------------------------------------------
Displacements are stride-2 in both axes, so f1 column j only ever
correlates with f2 columns of the SAME parity: the per-row-pair
all-pairs block is 32x32 per parity instead of 64x64 (2x less PE /
copy / DMA than the unsplit scheme).

Sharding: 8 cores = (row parity rp) x (col parity cp) x (half h).
Each core holds 12 f2 rows of parity rp at cp columns, as 3 stationary
tiles ("slots") of 4 rows x 32 cols.  The y-displacement window |r2-i|
<= 20 means slot quads near the volume edge need only 14/18/22 of the
24 same-parity f1 rows; with h=1 cores taking mirrored quads (and f1
rows stored reversed), the slot windows are uniformly 448/576/704
moving columns across all cores -- a single SPMD program computes only
the needed band (1728 instead of 2304 columns).

Scheduling (all latency-bound; every choice paired-A/B-benched on HW):
  - inputs on the two HWDGE rings: aux (= f2 stationary ++ f1 tail) on
    SP (its SDMA starts ~0.5us before ACT's, which is busy with the
    auto-inserted ACT table load), f1 head on ACT.
  - matmul order: the small tail chunks (B1, C1) are gated by aux and
    run in the ~0.5us before the f1 head lands, also warming the PE;
    then C0, B0, A -- the smallest slot (A) last keeps the critical
    tail (cast + DMA issue + ~1.4us HBM-write receipt) short.
  - one full-tile PSUM->SBUF cast per slot, DVE for C/A, ACT for B
    (chunked casts lose ~0.4us fixed overhead per op; matmuls are
    ISA-capped at 512 output columns).
  - two output DMAs: C on ACT, B+A merged on SP (SP is free after its
    input issue while ACT is busy with castB, so the last-gated DMA
    issues ~0.1us earlier); the output stream is HBM-write-bound
    (~220 GB/s aggregate; more queues don't help).
  - no PE warm-up matmuls: the HAM clock ramp needs ~5us of busy PE,
    which a ~2us kernel can never amortize (benched).

The band/diagonal gather and zero-fill is a pure data rearrangement
done on host during unsharding -- all arithmetic happens on device.
"""

import sys
import types

for _p in ("/opt/trn_rl_repo", "/root/.axon_site"):
    if _p not in sys.path:
        sys.path.insert(0, _p)

import ml_dtypes
import numpy as np

BF16 = ml_dtypes.bfloat16

import concourse.bacc as bacc
import concourse.mybir as mybir
from concourse import tile
from concourse import bass_utils
from concourse.bass_utils import run_bass_kernel_spmd

C = 128
H = 48
W = 64
D = 20
ND = 21            # displacements per axis
NCORES = 8
SLOT_COLS = (448, 576, 704)   # moving cols per stationary slot (A, B, C)
SLOT_OFF = (0, 448, 1024)
TOT = 1728                    # total output columns per core
F1COLS = 704                  # f1 moving columns per core (22 rows x 32)


def _ensure_ntff_hook():
    """Register the axon NTFF profile hook if possible (for trace runs)."""
    try:
        import antenv
        if "antenv.axon_hooks" not in sys.modules:
            mod = types.ModuleType("antenv.axon_hooks")
            _h = [None]
            mod.set_axon_ntff_profile_hook = lambda h: _h.__setitem__(0, h)
            mod.get_axon_ntff_profile_hook = lambda: _h[0]
            sys.modules["antenv.axon_hooks"] = mod
            antenv.axon_hooks = mod
        bass_utils.upload_artifacts = lambda tmpdir: "local://" + tmpdir
        from trn_agent_boot.trn_boot import _ntff_profile_via_ctypes
        sys.modules["antenv.axon_hooks"].set_axon_ntff_profile_hook(
            _ntff_profile_via_ctypes("/opt/axon/libaxon_pjrt.so")
        )
    except Exception:
        pass


def build_program():
    nc = bacc.Bacc(None, target_bir_lowering=False)
    aux = nc.declare_dram_parameter("aux", [C, 576], mybir.dt.bfloat16, isOutput=False)
    f1a = nc.declare_dram_parameter("f1a", [C, 512], mybir.dt.bfloat16, isOutput=False)
    mout = nc.declare_dram_parameter("mout", [C, TOT], mybir.dt.bfloat16, isOutput=True)

    with tile.TileContext(nc) as tc:
        with (
            tc.tile_pool(name="in", bufs=1) as in_pool,
            tc.tile_pool(name="msb", bufs=1) as m_pool,
            tc.tile_pool(name="ps", bufs=1, space="PSUM") as ps_pool,
        ):
            aux_sb = in_pool.tile([C, 576], mybir.dt.bfloat16, tag="aux")
            nc.sync.dma_start(out=aux_sb[:], in_=aux[:])
            f1a_sb = in_pool.tile([C, 512], mybir.dt.bfloat16, tag="f1a")
            nc.scalar.dma_start(out=f1a_sb[:], in_=f1a[:])

            def lhsT(x):
                return aux_sb[:, 128 * x : 128 * (x + 1)]

            f1b_sb = aux_sb

            psA = ps_pool.tile([128, 448], mybir.dt.float32, tag="psA")
            psB = ps_pool.tile([128, 576], mybir.dt.float32, tag="psB")
            psC = ps_pool.tile([128, 704], mybir.dt.float32, tag="psC")

            # the small tail matmuls (B1, C1) are gated by aux, which lands
            # ~0.5us before f1a -- run them FIRST to use that gap (and warm
            # the PE), then the f1a-gated big chunks
            nc.tensor.matmul(
                psB[:, 512:576], lhsT(1), f1b_sb[:, 384:448], start=True, stop=True
            )
            nc.tensor.matmul(
                psC[:, 512:704], lhsT(2), f1b_sb[:, 384:576], start=True, stop=True
            )
            nc.tensor.matmul(psC[:, :512], lhsT(2), f1a_sb[:], start=True, stop=True)
            mC = m_pool.tile([128, 704], mybir.dt.bfloat16, tag="mC")
            nc.vector.tensor_copy(mC[:], psC[:])
            nc.scalar.dma_start(out=mout[:, 1024:1728], in_=mC[:])

            nc.tensor.matmul(psB[:, :512], lhsT(1), f1a_sb[:], start=True, stop=True)
            mBA = m_pool.tile([128, 1024], mybir.dt.bfloat16, tag="mBA")
            nc.scalar.copy(mBA[:, 448:1024], psB[:])

            nc.tensor.matmul(psA[:, :448], lhsT(0), f1a_sb[:, :448], start=True, stop=True)
            nc.vector.tensor_copy(mBA[:, :448], psA[:])
            nc.sync.dma_start(out=mout[:, 0:1024], in_=mBA[:])
    nc.compile()
    return nc


_PROGRAM_CACHE = {}


def _get_program():
    if "nc" not in _PROGRAM_CACHE:
        _PROGRAM_CACHE["nc"] = build_program()
    return _PROGRAM_CACHE["nc"]


def _core_def(m):
    """Core m = rp*4 + cp*2 + h -> (rp, cp, h, f2 rows R, f1 rows I, cols J)."""
    rp, cp, h = m // 4, (m // 2) % 2, m % 2
    gs = (0, 1, 2) if h == 0 else (5, 4, 3)
    R = [rp + 8 * g + 2 * rq for g in gs for rq in range(4)]
    S = range(0, 22) if h == 0 else range(23, 1, -1)
    I = [rp + 2 * s for s in S]
    J = [cp + 2 * u for u in range(32)]
    return rp, cp, h, R, I, J


def _shard_inputs(features_1, features_2):
    f1 = np.ascontiguousarray(features_1, dtype=np.float32)
    f2 = np.ascontiguousarray(features_2, dtype=np.float32)
    in_maps = []
    for m in range(NCORES):
        rp, cp, h, R, I, J = _core_def(m)
        f2g = f2[:, R][:, :, J].reshape(C, 384)
        f1g = f1[:, I][:, :, J].reshape(C, F1COLS)
        aux = np.concatenate([f2g, f1g[:, 512:]], axis=1)
        in_maps.append(
            {
                "aux": np.ascontiguousarray(aux).astype(BF16),
                "f1a": np.ascontiguousarray(f1g[:, :512]).astype(BF16),
            }
        )
    return in_maps


def _assemble(results):
    """Gather the stride-2 displacement band out of the per-core blocks
    (pure indexing -- no arithmetic)."""
    M = np.stack(
        [np.asarray(results[m]["mout"]).astype(np.float32) for m in range(NCORES)]
    )  # [8, 128, 1728]

    dy, dxi, i, j = np.ogrid[0:ND, 0:ND, 0:H, 0:W]
    rp = i & 1
    cp = j & 1
    u = j >> 1
    r2 = i + 2 * dy - 20
    v = u + dxi - 10
    valid = (r2 >= 0) & (r2 < H) & (v >= 0) & (v < 32)
    r2c = np.clip(r2, 0, H - 1)
    vc = np.clip(v, 0, 31)
    k = (r2c - rp) >> 1
    g = k // 4
    rq = k % 4
    h = (g >= 3).astype(int)
    x = np.where(h == 0, g, 5 - g)
    s = (i - rp) >> 1
    l = np.where(h == 0, s, 23 - s)
    off = np.array(SLOT_OFF)[x]
    m = rp * 4 + cp * 2 + h
    part = 32 * rq + vc
    col = off + 32 * l + u
    bm, bp, bc = np.broadcast_arrays(m, part, col)
    out = M[bm, bp, bc]
    out[~np.broadcast_to(valid, out.shape)] = 0.0
    return out.reshape(1, ND * ND, H, W)


def kernel(features_1, features_2):
    nc = _get_program()
    in_maps = _shard_inputs(features_1, features_2)
    res = run_bass_kernel_spmd(nc, in_maps, list(range(NCORES)))
    return _assemble(res.results)


def kernel_traced(features_1, features_2, tmpdir=None):
    """Same as kernel() but with NTFF profiling; returns (output, exec_time_ns)."""
    _ensure_ntff_hook()
    nc = _get_program()
    in_maps = _shard_inputs(features_1, features_2)
    res = run_bass_kernel_spmd(
        nc, in_maps, list(range(NCORES)), trace=True, tmpdir=tmpdir
    )
    return _assemble(res.results), res.exec_time_ns
